# revision 10
# baseline (speedup 1.0000x reference)
"""Trainium2 Bass kernel for nn_MemristorConv2d_42494406427033.

Strategy
--------
Data-parallel over batch: 16 images / 8 cores = 2 images per core.
Weights (27 bit/tap matrices), bias and scalars replicated.

Everything stays in x's native [C, F, T] layout (f-major raster). With
W[c, f', t'] = padded fv image, the reference output is
    Y[b,o,fp,tp] = sum_{c,i,j} W[c, fp+j, tp+i] * g[bit][o,c,i,j]
so each 3x3 tap is a [K=C=128] x [M=O=128] x [N=512] matmul on a shifted
window of the padded image, accumulated in PSUM, and the result raster
[o, fp*64+tp] DMAs contiguously into the output. No transposes anywhere.

DAC round / ADC round use the fp32 RNE magic-number trick (+1.5*2^23).
The ADC's clip(+-16) provably never binds (|outs| <= 1152*0.6216 < 1600),
and the 2.56 ADC prescale is folded into the conv weights. The final
combine is  out = (2*R0 + R1 + R2) * (output_factor/128) + bias  which is
bit-exact with the reference's bit-accumulation (all R_i integer-valued).

Matmuls run as float32r (full PE rate at free-dim 512).
"""

import os
import sys

import numpy as np

for _p in ("/opt/trn_rl_repo", "/root/.axon_site/_ro/trn_rl_repo"):
    if os.path.isdir(_p) and _p not in sys.path:
        sys.path.insert(0, _p)

import concourse.bass as bass
import concourse.bacc as bacc
import concourse.tile as tile
from concourse import mybir
from concourse.bass_utils import run_bass_kernel_spmd

F32 = mybir.dt.float32
F32R = mybir.dt.float32r
AF = mybir.ActivationFunctionType
OP = mybir.AluOpType

B, C, O, F, T = 16, 128, 128, 64, 64
NCORES = 8
BPC = B // NCORES          # images per core
PW = F + 2                 # padded side 66
NPAD = PW * PW             # 4356
NPIX = F * T               # 4096
FT = 8                     # f-rows per output tile -> free dim 512
NT = F // FT               # 8 output tiles per image
MAGIC = 12582912.0         # 1.5 * 2**23, fp32 RNE round-to-integer bias
C1 = float(np.float32(1.0) / np.float32(127.0))

_NC_CACHE = {}


def _build_nc():
    nc = bacc.Bacc()
    xs = nc.declare_dram_parameter("xs", [BPC, C, NPIX], F32, isOutput=False)
    wd = nc.declare_dram_parameter("wt", [C, 27 * O], F32R, isOutput=False)
    sc = nc.declare_dram_parameter("scal", [C, 4], F32, isOutput=False)
    outd = nc.declare_dram_parameter("out", [BPC, O, NPIX], F32, isOutput=True)

    from contextlib import ExitStack

    with tile.TileContext(nc) as tc, ExitStack() as ctx:
        constp = ctx.enter_context(tc.tile_pool(name="const", bufs=1))
        big = ctx.enter_context(tc.tile_pool(name="big", bufs=3))
        tp = ctx.enter_context(tc.tile_pool(name="tp", bufs=2))
        fvp = ctx.enter_context(tc.tile_pool(name="fvp", bufs=2))
        adcp = ctx.enter_context(tc.tile_pool(name="adcp", bufs=6))
        outp = ctx.enter_context(tc.tile_pool(name="outp", bufs=3))
        psp = ctx.enter_context(tc.tile_pool(name="psum", bufs=6, space="PSUM"))

        wt = constp.tile([C, 27 * O], F32R)
        nc.gpsimd.dma_start(out=wt[:], in_=wd[:])
        sct = constp.tile([C, 4], F32)
        nc.gpsimd.dma_start(out=sct[:], in_=sc[:])
        ifs, scf, bi = sct[:, 0:1], sct[:, 1:2], sct[:, 2:3]

        for img in range(BPC):
            xv = big.tile([C, NPIX], F32)
            nc.gpsimd.dma_start(out=xv[:], in_=xs[img])

            # DAC: r = round(clip(x*input_factor, -1, 1) * 127)
            nc.vector.tensor_scalar(xv[:], xv[:], ifs, 1.0, op0=OP.mult, op1=OP.min)
            nc.vector.tensor_scalar(xv[:], xv[:], -1.0, 127.0, op0=OP.max, op1=OP.mult)
            nc.scalar.activation(xv[:], xv[:], AF.Copy, bias=MAGIC)
            t = tp.tile([C, NPIX], F32)
            nc.vector.tensor_scalar(t[:], xv[:], MAGIC, C1, op0=OP.subtract, op1=OP.mult)
            nc.scalar.activation(t[:], t[:], AF.Copy, scale=0.6)
            # fv = v + 0.1*v^3 into the padded f32r buffer (zeroed borders)
            s = big.tile([C, NPIX], F32)
            nc.vector.tensor_tensor(s[:], t[:], t[:], op=OP.mult)
            nc.vector.tensor_tensor(s[:], s[:], t[:], op=OP.mult)
            fv = fvp.tile([C, NPAD], F32R)
            fv3 = fv[:].rearrange("p (a b) -> p a b", b=PW)
            fz = fv[:].bitcast(F32).rearrange("p (a b) -> p a b", b=PW)
            nc.vector.memset(fz[:, 0, :], 0.0)
            nc.vector.memset(fz[:, PW - 1, :], 0.0)
            nc.vector.memset(fz[:, 1 : PW - 1, 0], 0.0)
            nc.vector.memset(fz[:, 1 : PW - 1, PW - 1], 0.0)
            s3 = s[:].rearrange("p (a b) -> p a b", b=F)
            t3 = t[:].rearrange("p (a b) -> p a b", b=F)
            nc.vector.scalar_tensor_tensor(
                fv3[:, 1 : PW - 1, 1 : PW - 1], s3, 0.1, t3, op0=OP.mult, op1=OP.add
            )
            for pt in range(NT):
                f0 = pt * FT
                a_list = []
                for i in range(3):
                    ps = psp.tile([O, FT * T], F32)
                    for y in range(3):
                        for xk in range(3):
                            k = i * 9 + y * 3 + xk
                            rhs = fv3[:, f0 + xk : f0 + xk + FT, y : y + T]
                            nc.tensor.matmul(
                                ps[:],
                                wt[:, k * O : (k + 1) * O],
                                rhs,
                                start=(y == 0 and xk == 0),
                                stop=(y == 2 and xk == 2),
                            )
                    a = adcp.tile([O, FT * T], F32)
                    # a_i = R_i + MAGIC  (RNE round of psum happens in this add)
                    nc.scalar.activation(a[:], ps[:], AF.Copy, bias=MAGIC)
                    a_list.append(a)
                a0, a1, a2 = a_list
                u = outp.tile([O, FT * T], F32)
                nc.vector.tensor_scalar(u[:], a0[:], MAGIC, 2.0, op0=OP.subtract, op1=OP.mult)
                nc.vector.scalar_tensor_tensor(u[:], a1[:], MAGIC, u[:], op0=OP.subtract, op1=OP.add)
                nc.vector.scalar_tensor_tensor(u[:], a2[:], MAGIC, u[:], op0=OP.subtract, op1=OP.add)
                # out = u * (output_factor/128) + bias
                nc.vector.tensor_scalar(u[:], u[:], scf, bi, op0=OP.mult, op1=OP.add)
                nc.gpsimd.dma_start(out=outd[img][:, f0 * T : (f0 + FT) * T], in_=u[:])
    nc.compile()
    return nc


def _prep_inputs(x, g_pos, g_neg, bias, input_factor, output_factor):
    xf = np.ascontiguousarray(np.asarray(x, dtype=np.float32)).reshape(B, C, NPIX)
    gs = (np.asarray(g_pos, np.float32) - np.asarray(g_neg, np.float32)) * np.float32(2.56)
    # [bit,o,c,i,j] -> [c, bit,i,j, o] -> [C, 27*O]
    W = np.ascontiguousarray(np.transpose(gs, (2, 0, 3, 4, 1)).reshape(C, 27 * O))
    scal = np.zeros((C, 4), np.float32)
    scal[:, 0] = np.float32(input_factor)
    scal[:, 1] = np.float32(output_factor) / np.float32(128.0)  # exact power-of-2 scale
    scal[:, 2] = np.asarray(bias, np.float32)
    in_maps = [
        {"xs": xf[k * BPC : (k + 1) * BPC], "wt": W, "scal": scal}
        for k in range(NCORES)
    ]
    return in_maps


def _get_nc():
    if "nc" not in _NC_CACHE:
        _NC_CACHE["nc"] = _build_nc()
    return _NC_CACHE["nc"]


def run(inputs, trace=False):
    """Run on 8 NeuronCores. Returns (full_output, BassKernelResults)."""
    nc = _get_nc()
    in_maps = _prep_inputs(**inputs)
    res = run_bass_kernel_spmd(nc, in_maps, list(range(NCORES)), trace=trace)
    out = np.concatenate(
        [np.asarray(res.results[k]["out"]).reshape(BPC, O, F, T) for k in range(NCORES)],
        axis=0,
    )
    return out, res


def kernel(**inputs):
    out, _ = run(inputs)
    return out


# revision 11
# speedup vs baseline: 1.2021x; 1.2021x over previous
"""Trainium2 Bass kernel for nn_MemristorConv2d_42494406427033.

Strategy
--------
Data-parallel over batch: 16 images / 8 cores = 2 images per core.
Weights (27 bit/tap matrices), bias and scalars replicated.

Everything stays in x's native [C, F, T] layout (f-major raster). With
W[c, f', t'] = padded fv image, the reference output is
    Y[b,o,fp,tp] = sum_{c,i,j} W[c, fp+j, tp+i] * g[bit][o,c,i,j]
so each 3x3 tap is a [K=C=128] x [M=O=128] x [N=512] matmul on a shifted
window of the padded image, accumulated in PSUM, and the result raster
[o, fp*64+tp] DMAs contiguously into the output. No transposes anywhere.

DAC round / ADC round use the fp32 RNE magic-number trick (+1.5*2^23).
The ADC's clip(+-16) provably never binds (|outs| <= 1152*0.6216 < 1600),
and the 2.56 ADC prescale is folded into the conv weights. The final
combine is  out = (2*R0 + R1 + R2) * (output_factor/128) + bias  which is
bit-exact with the reference's bit-accumulation (all R_i integer-valued).

Pipelining: the DAC chain runs in 4 chunks of 16 f-rows per image so the
first pixel-tile matmuls start after ~1 chunk instead of the whole image.
Each pixel tile accumulates its 3 bit-planes into one 3-bank PSUM tile,
rounded by a single ACT op; pixel tiles are paired so the bit-combine runs
as width-1024 DVE ops and the output DMA is one [128,1024] transfer.

Matmuls run as float32r (full PE rate at free-dim 512).
"""

import os
import sys

import numpy as np

for _p in ("/opt/trn_rl_repo", "/root/.axon_site/_ro/trn_rl_repo"):
    if os.path.isdir(_p) and _p not in sys.path:
        sys.path.insert(0, _p)

import concourse.bass as bass
import concourse.bacc as bacc
import concourse.tile as tile
from concourse import mybir
from concourse.bass_utils import run_bass_kernel_spmd

F32 = mybir.dt.float32
F32R = mybir.dt.float32r
AF = mybir.ActivationFunctionType
OP = mybir.AluOpType

B, C, O, F, T = 16, 128, 128, 64, 64
NCORES = 8
BPC = B // NCORES          # images per core
PW = F + 2                 # padded side 66
NPAD = PW * PW             # 4356
NPIX = F * T               # 4096
FT = 8                     # f-rows per output tile -> free dim 512
NT = F // FT               # 8 output tiles per image
GCH = 4                    # DAC chunks per image
CHR = F // GCH             # f-rows per chunk (16)
CHW = CHR * T              # elements per chunk per partition (1024)
MAGIC = 12582912.0         # 1.5 * 2**23, fp32 RNE round-to-integer bias
C059 = float(np.float32(np.float32(1.0) / np.float32(127.0)) * np.float32(0.6))

_NC_CACHE = {}


def _build_nc():
    nc = bacc.Bacc()
    xs = nc.declare_dram_parameter("xs", [BPC, C, NPIX], F32, isOutput=False)
    wd = nc.declare_dram_parameter("wt", [C, 27 * O], F32R, isOutput=False)
    sc = nc.declare_dram_parameter("scal", [C, 4], F32, isOutput=False)
    outd = nc.declare_dram_parameter("out", [BPC, O, NPIX], F32, isOutput=True)

    from contextlib import ExitStack

    with tile.TileContext(nc) as tc, ExitStack() as ctx:
        constp = ctx.enter_context(tc.tile_pool(name="const", bufs=1))
        big = ctx.enter_context(tc.tile_pool(name="big", bufs=2))
        chp = ctx.enter_context(tc.tile_pool(name="chp", bufs=3))
        fvp = ctx.enter_context(tc.tile_pool(name="fvp", bufs=2))
        adcp = ctx.enter_context(tc.tile_pool(name="adcp", bufs=2))
        outp = ctx.enter_context(tc.tile_pool(name="outp", bufs=3))
        psp = ctx.enter_context(tc.tile_pool(name="psum", bufs=2, space="PSUM"))

        wt = constp.tile([C, 27 * O], F32R)
        nc.gpsimd.dma_start(out=wt[:], in_=wd[:])
        sct = constp.tile([C, 4], F32)
        nc.gpsimd.dma_start(out=sct[:], in_=sc[:])
        ifs, scf, bi = sct[:, 0:1], sct[:, 1:2], sct[:, 2:3]

        for img in range(BPC):
            xv = big.tile([C, NPIX], F32)
            nc.gpsimd.dma_start(out=xv[:], in_=xs[img])

            fv = fvp.tile([C, NPAD], F32R)
            fv3 = fv[:].rearrange("p (a b) -> p a b", b=PW)
            fz = fv[:].bitcast(F32).rearrange("p (a b) -> p a b", b=PW)
            nc.gpsimd.memset(fz[:, 0, :], 0.0)
            nc.gpsimd.memset(fz[:, PW - 1, :], 0.0)
            nc.gpsimd.memset(fz[:, 1 : PW - 1, 0], 0.0)
            nc.gpsimd.memset(fz[:, 1 : PW - 1, PW - 1], 0.0)

            # DAC + memristor I-V, pipelined in chunks of CHR f-rows:
            #   r = round(clip(x*if,-1,1)*127); v = r*(0.6/127);
            #   fv = v*(1 + 0.1*v^2)
            for g in range(GCH):
                sl = slice(g * CHW, (g + 1) * CHW)
                cvt = chp.tile([C, CHW], F32, tag="cv")
                nc.vector.tensor_scalar(cvt[:], xv[:, sl], ifs, 1.0, op0=OP.mult, op1=OP.min)
                nc.vector.tensor_scalar(cvt[:], cvt[:], -1.0, None, op0=OP.max)
                nc.scalar.activation(cvt[:], cvt[:], AF.Copy, bias=MAGIC, scale=127.0)
                nc.vector.tensor_scalar(cvt[:], cvt[:], MAGIC, C059, op0=OP.subtract, op1=OP.mult)
                sq = chp.tile([C, CHW], F32, tag="sq")
                nc.scalar.activation(sq[:], cvt[:], AF.Square)
                nc.vector.tensor_scalar(sq[:], sq[:], 0.1, 1.0, op0=OP.mult, op1=OP.add)
                dst = fv3[:, 1 + g * CHR : 1 + (g + 1) * CHR, 1 : PW - 1]
                c3 = cvt[:].rearrange("p (a b) -> p a b", b=T)
                q3 = sq[:].rearrange("p (a b) -> p a b", b=T)
                nc.vector.tensor_tensor(dst, q3, c3, op=OP.mult)

            # Conv + ADC per pixel tile; pixel tiles paired for the combine.
            for pair in range(NT // 2):
                # a layout: [bit i][pair slot j][512] at offset i*1024 + j*512
                a = adcp.tile([O, 2 * 3 * FT * T], F32)
                av = a[:].rearrange("p (i j q) -> p i j q", i=3, j=2)
                for j in range(2):
                    pt = 2 * pair + j
                    f0 = pt * FT
                    ps = psp.tile([O, 3 * FT * T], F32)
                    for i in range(3):
                        for y in range(3):
                            for xk in range(3):
                                k = i * 9 + y * 3 + xk
                                rhs = fv3[:, f0 + xk : f0 + xk + FT, y : y + T]
                                nc.tensor.matmul(
                                    ps[:, i * 512 : (i + 1) * 512],
                                    wt[:, k * O : (k + 1) * O],
                                    rhs,
                                    start=(y == 0 and xk == 0),
                                    stop=(y == 2 and xk == 2),
                                )
                    # round all 3 bit-planes of this pixel tile in one op
                    p3 = ps[:].rearrange("p (i q) -> p i q", q=512)
                    nc.scalar.activation(av[:, :, j, :], p3, AF.Copy, bias=MAGIC)
                u = outp.tile([O, 2 * FT * T], F32)
                nc.vector.tensor_scalar(u[:], a[:, 0:1024], MAGIC, 2.0, op0=OP.subtract, op1=OP.mult)
                nc.vector.scalar_tensor_tensor(u[:], a[:, 1024:2048], MAGIC, u[:], op0=OP.subtract, op1=OP.add)
                nc.vector.scalar_tensor_tensor(u[:], a[:, 2048:3072], MAGIC, u[:], op0=OP.subtract, op1=OP.add)
                nc.vector.tensor_scalar(u[:], u[:], scf, bi, op0=OP.mult, op1=OP.add)
                nc.gpsimd.dma_start(
                    out=outd[img][:, pair * 2 * FT * T : (pair + 1) * 2 * FT * T], in_=u[:]
                )
    nc.compile()
    return nc


def _prep_inputs(x, g_pos, g_neg, bias, input_factor, output_factor):
    xf = np.ascontiguousarray(np.asarray(x, dtype=np.float32)).reshape(B, C, NPIX)
    gs = (np.asarray(g_pos, np.float32) - np.asarray(g_neg, np.float32)) * np.float32(2.56)
    # [bit,o,c,i,j] -> [c, bit,i,j, o] -> [C, 27*O]
    W = np.ascontiguousarray(np.transpose(gs, (2, 0, 3, 4, 1)).reshape(C, 27 * O))
    scal = np.zeros((C, 4), np.float32)
    scal[:, 0] = np.float32(input_factor)
    scal[:, 1] = np.float32(output_factor) / np.float32(128.0)  # exact power-of-2 scale
    scal[:, 2] = np.asarray(bias, np.float32)
    in_maps = [
        {"xs": xf[k * BPC : (k + 1) * BPC], "wt": W, "scal": scal}
        for k in range(NCORES)
    ]
    return in_maps


def _get_nc():
    if "nc" not in _NC_CACHE:
        _NC_CACHE["nc"] = _build_nc()
    return _NC_CACHE["nc"]


def run(inputs, trace=False):
    """Run on 8 NeuronCores. Returns (full_output, BassKernelResults)."""
    nc = _get_nc()
    in_maps = _prep_inputs(**inputs)
    res = run_bass_kernel_spmd(nc, in_maps, list(range(NCORES)), trace=trace)
    out = np.concatenate(
        [np.asarray(res.results[k]["out"]).reshape(BPC, O, F, T) for k in range(NCORES)],
        axis=0,
    )
    return out, res


def kernel(**inputs):
    out, _ = run(inputs)
    return out
